# revision 48
# baseline (speedup 1.0000x reference)
"""Multi-head causal attention (RoPE) forward on 8 Trainium2 NeuronCores.

Sharding: tensor-parallel over heads -- 8 cores x 2 heads, each core handling
both batch elements (the flattened (B*T) = 4096 "time" axis). All matmul
operands are fp16 (PSUM accumulation stays fp32; measured end-to-end rel err
~4e-4 against a 2e-2 gate; fp8 was measured to fail it). Inputs are
host-retiled so every DMA moves contiguous per-partition runs.

Per core:
  merged phase: q/k/v projections in eight 512-column t-sections. RoPE is
      praw*cos + shuffle(praw)*sin with the pair-swap done by a DVE
      stream_shuffle and the signs folded into the host sin table (no PE
      work). Head-0's attention chunk (b, ci) is interleaved right after
      the section that completes its k/q/v, sharing PSUM by tag
      (proj/scores 5 bufs, out-accum 1, v/denominator 2 = 8 banks), so the
      first AllToAll fires at projection end.
  attention (per chunk): scores^T [j, i] = kT^T @ qT with causal
      column-support pruning (diagonal tiles compute only their valid
      column suffix); exp on ScalarE (no max pass -- scores are bounded),
      partial tiles masked by a 0/1 multiply after exp; out^T accumulates
      on PE. The softmax denominator differs by head: head-0 chunks (in
      phase 1, where DVE has slack) accumulate probability tiles on DVE
      and reduce with ONE all-ones ones-matmul per chunk (512 PE rows vs
      ~4.4k); head-1 chunks (under the first AllToAll, where DVE is the
      scarce engine) pair up full-width tiles with one DVE add each so
      the ones-matmul streams once per pair (diagonal tiles stay
      per-tile) -- full accumulation there saturated the DVE and delayed
      the second AllToAll, pairing fits with ~8us DVE slack to spare.
      Normalization uses a fast approximate reciprocal. A
      4-deep score-matmul software pipeline keeps the PE ahead of the exp
      chain. Head-1 runs after the projections largest-chunk-first (the
      last chunk's tail gates the second AllToAll trigger), covering the
      first AllToAll; a tiny warmup collective during the projections
      absorbs the one-time ring setup.
  output projection: y[t-slice, :] = outT_full^T @ wo in two full passes
      split by head parity -- y_g0 (gated only on the first AllToAll)
      computes into SBUF during the second collective's window, then y_g1
      accumulates and adds it back. The first column block of each stage
      runs contraction-group-outer so the PE starts on the first gathered
      tile that lands. y is stored fp16 (host upcasts; ~5e-4 rel rounding
      vs the 2e-2 gate) so the final store drain is ~3us, not ~15us.
Host assembles the 8 t-slices into the full (B, T, C) output.

Hard-won scheduling rules (measured on hardware):
- DMA trigger issue costs ~600ns on an engine queue; only sync/scalar/
  gpsimd may issue them. A trigger whose semaphore waits on a collective
  must NOT sit on a queue with earlier-needed compute: the tile scheduler
  can hoist it and the wait then stalls that engine (7.6us PE stall via a
  blocked ACT exp queue when ao-loads used nc.scalar).
- All DMAs use flattened 2D access patterns (free axis pre-flattened) so
  descriptors carry 2KB+ per-partition runs; 1KB descriptors cap each
  queue at ~10GB/s.
- Do NOT offload bulk tensor ops to gpsimd or oversubscribe DVE: with
  PE+ACT+DVE+gpsimd all hot, every engine slowed ~20% chip-wide
  (power/throttle) and the kernel regressed 25us.
- The dma_start count/order re-rolls the global queue assignment; timings
  move +-5us run to run from this lottery alone.
"""

import os
import sys

import numpy as np

for _p in ("/opt/trn_rl_repo", "/root/.axon_site/_ro/trn_rl_repo"):
    if os.path.isdir(_p) and _p not in sys.path:
        sys.path.append(_p)

import concourse.bacc as bacc
import concourse.tile as tile
from concourse import mybir
from concourse.bass_utils import run_bass_kernel_spmd

B, T, C = 2, 2048, 2048
N_HEADS, D = 16, 128
THETA = 10000.0
N_CORES = 8
HPC = N_HEADS // N_CORES     # heads per core
BT = B * T                   # flattened time axis
TSL = BT // N_CORES          # per-core output slice after the all-to-all
KT = C // 128                # contraction chunks
TC1 = 512                    # phase-1 t-chunk (moving free dim)
NTC1 = BT // TC1
TC2 = 512                    # phase-2/3 chunk
CI = T // TC2                # i-chunks per (head, batch)
JT = T // 128                # j-tiles per (head, batch)
SCALE = 1.0 / np.sqrt(D)
MASKED_BIAS = -1.0e6         # pre-scale units; exp(SCALE*(s+bias)) == 0

F16 = mybir.dt.float16
F32 = mybir.dt.float32
# adjacent-pair swap within each 32-partition quadrant (RoPE rotate-half)
SWAP_MASK = [i + 1 if i % 2 == 0 else i - 1 for i in range(32)]


def _mask_plan(mask2d):
    """Per (ci, jt): None=skip (all masked), (-1, 0)=free (none masked),
    (idx, lo) = partial tile whose columns [lo, TC2) have at least one valid
    row; idx is a 0/1 multiplier tile applied to exp(scores) on that column
    range. scoresT tile (jt, ci) holds mask2d[i, j] transposed:
    m01[j_loc, i_loc] <- mask2d[TC2*ci+i, 128*jt+j].
    """
    uniq = {}
    tiles = []
    plan = []
    for ci in range(CI):
        row = []
        for jt in range(JT):
            blk = mask2d[TC2 * ci:TC2 * (ci + 1), 128 * jt:128 * (jt + 1)]
            if blk.all():
                row.append((-1, 0))
            elif not blk.any():
                row.append(None)
            else:
                m01 = blk.T.astype(np.float16)   # [128 j, TC2 i]
                lo = int(np.argmax(m01.any(axis=0)))
                key = m01.tobytes()
                if key not in uniq:
                    uniq[key] = len(tiles)
                    tiles.append(m01)
                row.append((uniq[key], lo))
        plan.append(row)
    if not tiles:  # keep the DRAM tensor non-empty
        tiles.append(np.zeros((128, TC2), np.float16))
    return plan, np.stack(tiles)


def _rope_tables():
    inv_freq = 1.0 / (THETA ** (np.arange(0, D, 2, dtype=np.float64) / D))
    freqs = np.outer(inv_freq, np.arange(T, dtype=np.float64))  # [64, T]
    cosI = np.repeat(np.cos(freqs), 2, axis=0).astype(np.float16)  # [128, T]
    # the pair-swap runs as a signless DVE stream_shuffle, so the RoPE signs
    # ride in the sin table: row 2i gets -sin (multiplies x[2i+1]), row
    # 2i+1 gets +sin (multiplies x[2i])
    sinI = np.repeat(np.sin(freqs), 2, axis=0).astype(np.float16)
    sinI[0::2, :] *= np.float16(-1.0)
    return cosI, sinI


def _phase12(nc, tc, qkv_tensors, xTt, cos_sb, sin_sb, late_loads,
             plan, n_bias, bias_d, ones_sb, a2a_in, a2a_out, qkv, wot, wha):
    """Projections with head-0 attention chunks interleaved: chunk
    (h=0, b, ci) only needs k/q/v through t-chunk 4*b+ci, so it runs right
    after that section and the first all-to-all fires at projection end
    instead of mid-attention. PSUM is shared by tag: proj/sc (4 bufs),
    rot/outT (2), vps/r (2) = exactly 8 banks."""
    qT, kT, vt, wq_h, wk_h, wv_sb = qkv_tensors
    with tc.tile_pool(name="xt", bufs=2) as xp, \
         tc.tile_pool(name="p1t", bufs=1) as p1, \
         tc.tile_pool(name="p2t", bufs=1) as p2, \
         tc.tile_pool(name="ps12", bufs=1, space="PSUM") as pp:
        mask_sb = p2.tile([128, n_bias, TC2], F16)
        LA = 4   # sc-matmul lookahead so the PE never waits on ACT's exp

        def emit_chunk(h, b, ci):
            gci = b * CI + ci      # global chunk == dest rank
            live = [(jt, plan[ci][jt]) for jt in range(JT)
                    if plan[ci][jt] is not None]
            if not live:
                z = p2.tile([128, TC2], F16, name="z", tag="ot", bufs=3)
                nc.vector.memset(z[:], 0.0)
                nc.sync.dma_start(a2a_in[h][gci, :, :], z[:])
                return
            # column-support pruning relies on the first live tile
            # starting the full accumulation region
            full_width = live[0][1][1] != 0
            outp = pp.tile([D, TC2], F32, name="outp", tag="mid", bufs=1)
            i0 = b * T + ci * TC2
            nlive = len(live)
            pend = [None] * nlive
            # Denominator, two flavors. Head-0 chunks run inside phase 1
            # where the DVE has slack: probability tiles accumulate on DVE
            # (fp16 adds) and ONE ones-matmul per chunk reduces over j
            # (512 PE rows instead of ~4.4k). Head-1 chunks run under the
            # first all-to-all where the DVE is the scarce engine and PE
            # has slack: keep the per-tile PE ones-matmul so the chunk
            # tails (-> ot DMA -> second all-to-all trigger) stay early.
            # (Tried: head-1 on DVE too, normalize on gpsimd -- every
            # engine slowed ~20% chip-wide and a2a#2 slipped 28us.)
            dve_acc = h == 0
            acc = None
            rp = pp.tile([128, TC2], F32, name="rp", tag="vr", bufs=2)
            if dve_acc:
                acc = p2.tile([128, TC2], F16, name="acc", tag="acc",
                              bufs=3)
            # head-1 middle ground: pair up the full-width tiles on DVE
            # (one add each) so the PE ones-matmul streams once per PAIR;
            # diagonal tiles keep the per-tile matmul. ~6us less PE in the
            # window that gates the second all-to-all, and the extra DVE
            # fits (unlike full accumulation, which saturated it).
            nf = 0
            if not dve_acc and not full_width:
                while (nf < nlive and live[nf][1][0] < 0
                       and live[nf][1][1] == 0):
                    nf += 1
                nf -= nf % 4   # quads only; leftovers take the diag path
            paccs = {}
            dpend = None
            for step in range(nlive + LA):
                if step < nlive:
                    jt, (code, lo) = live[step]
                    if full_width:
                        lo = 0
                    sc = pp.tile([128, TC2], F32, name="sc", tag="big",
                                 bufs=5)
                    nc.tensor.matmul(
                        sc[:, lo:],
                        kT[h][:, b * T + jt * 128:b * T + (jt + 1) * 128],
                        qT[h][:, i0 + lo:i0 + TC2], start=True, stop=True)
                    pt = p2.tile([128, TC2], F16, name="pt", tag="pt",
                                 bufs=5)
                    nc.scalar.activation(
                        pt[:, lo:], sc[:, lo:],
                        mybir.ActivationFunctionType.Exp,
                        bias=0.0, scale=float(SCALE))
                    if code >= 0:
                        ptm = p2.tile([128, TC2], F16, name="ptm",
                                      tag="ptm", bufs=5)
                        nc.vector.tensor_mul(ptm[:, lo:], pt[:, lo:],
                                             mask_sb[:, code, lo:])
                        pt = ptm
                    if dve_acc:
                        if step == 0:
                            nc.vector.tensor_copy(acc[:, lo:], pt[:, lo:])
                        else:
                            nc.vector.tensor_add(acc[:, lo:], acc[:, lo:],
                                                 pt[:, lo:])
                    elif step < nf and step % 2 == 1:
                        # quad tree on DVE: two pair adds + one in-place
                        # root add per 4 full tiles; the ring shares the
                        # (phase-disjoint) acc tag
                        pacc = p2.tile([128, TC2], F16, name="pacc",
                                       tag="acc", bufs=3)
                        nc.vector.tensor_add(pacc[:], pend[step - 1][0][:],
                                             pt[:])
                        if step % 4 == 1:
                            paccs[step - 1] = pacc
                        else:
                            root = paccs[step - 3]
                            nc.vector.tensor_add(root[:], root[:], pacc[:])
                    elif (nf > 0 and step > nf and (step - nf) % 2 == 1):
                        # diagonal tiles pair too (only when quads already
                        # start-initialized the full rp region): add on the
                        # overlap [lo_odd:); the even tile's uncovered
                        # prefix gets its own short matmul at the out step
                        dacc = p2.tile([128, TC2], F16, name="dacc",
                                       tag="acc", bufs=3)
                        nc.vector.tensor_add(dacc[:, lo:],
                                             pend[step - 1][0][:, lo:],
                                             pt[:, lo:])
                        paccs[step - 1] = (dacc, lo)
                    pend[step] = (pt, lo)
                j = step - LA
                if 0 <= j < nlive:
                    pt_j, lo_j = pend[j]
                    pend[j] = None
                    jv = (b * T) // 128 + live[j][0]
                    nc.tensor.matmul(
                        outp[:, lo_j:],
                        vt[jv][:, h * D:(h + 1) * D], pt_j[:, lo_j:],
                        start=(j == 0), stop=(j == nlive - 1))
                    if not dve_acc:
                        # ones_sb is [128, 128] all-ones: every partition
                        # of rp gets the denominator -- broadcast free on PE
                        if j < nf:
                            if j % 4 == 3:
                                nc.tensor.matmul(
                                    rp[:], ones_sb[:], paccs.pop(j - 3)[:],
                                    start=(j == 3), stop=(j == nlive - 1))
                        elif nf > 0 and (j - nf) % 2 == 0 and j + 1 < nlive:
                            dpend = (pt_j, lo_j)   # partner emits both mms
                        elif nf > 0 and (j - nf) % 2 == 1:
                            dacc, lo_o = paccs.pop(j - 1)
                            pt_e, lo_e = dpend
                            if lo_e < lo_o:
                                nc.tensor.matmul(
                                    rp[:, lo_e:lo_o], ones_sb[:],
                                    pt_e[:, lo_e:lo_o],
                                    start=False, stop=False)
                            nc.tensor.matmul(
                                rp[:, lo_o:], ones_sb[:], dacc[:, lo_o:],
                                start=False, stop=(j == nlive - 1))
                        else:
                            nc.tensor.matmul(
                                rp[:, lo_j:], ones_sb[:], pt_j[:, lo_j:],
                                start=(j == 0 and nf == 0),
                                stop=(j == nlive - 1))
            if dve_acc:
                nc.tensor.matmul(rp[:], ones_sb[:], acc[:],
                                 start=True, stop=True)
            ri = p2.tile([128, TC2], F32, name="ri", tag="ri", bufs=1)
            # ~18 correct bits, 5x faster than reciprocal(); r >= exp(s_ii)
            # here so the undefined edge cases cannot occur
            nc.vector.reciprocal_approx_fast(ri[:], rp[:])
            ot = p2.tile([128, TC2], F16, name="ot", tag="ot", bufs=3)
            nc.vector.tensor_mul(ot[:], outp[:], ri[:])
            # 4-way split: the last chunk's ot store gates the collective
            # trigger; 32KB per queue lands ~3us sooner than 64KB
            for p_ in range(4):
                nc.sync.dma_start(a2a_in[h][gci, 32 * p_:32 * (p_ + 1), :],
                                  ot[32 * p_:32 * (p_ + 1), :])

        # startup triggers spread over engines idle at t=0 (trigger issue
        # is ~600ns each; one queue serializes the whole first-tile load)
        eng0 = [nc.sync, nc.scalar, nc.gpsimd]
        for tcn in range(NTC1):
            ts = tcn * TC1           # position in flattened BT
            tp = ts % T              # rope position (restarts per batch)
            # flattened free axis: per-partition contiguous 2KB+ runs give
            # the DMA engines full-size descriptors instead of 1KB rows
            xt = xp.tile([128, KT * TC1], F16, tag="xt")
            nparts = 16 if tcn == 0 else 2
            step = (KT // nparts) * TC1
            for q_ in range(nparts):
                if tcn == 0:
                    for p_ in range(2):
                        eng0[(2 * q_ + p_) % len(eng0)].dma_start(
                            xt[64 * p_:64 * (p_ + 1),
                               q_ * step:(q_ + 1) * step],
                            xTt[tcn, 64 * p_:64 * (p_ + 1),
                                q_ * step:(q_ + 1) * step])
                else:
                    nc.sync.dma_start(xt[:, q_ * step:(q_ + 1) * step],
                                      xTt[tcn, :, q_ * step:(q_ + 1) * step])
            if tcn == 0:
                late_loads()     # weight DMAs behind the critical first loads
                nc.sync.dma_start(mask_sb[:],
                                  bias_d.rearrange("u p m -> p u m"))
            if tcn in (NTC1 // 2, NTC1 // 2 + 1):
                # prefetch the output-projection weights in the DMA-idle
                # mid-projection window, so no wo traffic contends with the
                # first all-to-all's wire phase
                for cj in (0, 1) if tcn == NTC1 // 2 else (2, 3):
                    wha[cj] = qkv.tile([128, KT * TC2], F16, name=f"wo{cj}")
                    stp = 4 * TC2
                    for q_ in range(4):
                        nc.sync.dma_start(
                            wha[cj][:, q_ * stp:(q_ + 1) * stp],
                            wot[cj, :, q_ * stp:(q_ + 1) * stp])
            for dst, w_h in ((qT, wq_h), (kT, wk_h)):
                for h in range(HPC):
                    ps = pp.tile([D, TC1], F32, name="ps", tag="big", bufs=5)
                    for cc in range(KT):
                        nc.tensor.matmul(
                            ps[:], w_h[h][:, cc * D:(cc + 1) * D],
                            xt[:, cc * TC1:(cc + 1) * TC1],
                            start=(cc == 0), stop=(cc == KT - 1))
                    praw = p1.tile([D, TC1], F16, tag="praw", bufs=3)
                    nc.scalar.copy(praw[:], ps[:])
                    rotc = p1.tile([D, TC1], F16, tag="rotc", bufs=2)
                    nc.vector.stream_shuffle(rotc[:], praw[:], SWAP_MASK)
                    t1 = p1.tile([D, TC1], F16, tag="t1", bufs=2)
                    nc.vector.tensor_mul(t1[:], praw[:], cos_sb[:, tp:tp + TC1])
                    t2 = p1.tile([D, TC1], F16, tag="t2", bufs=2)
                    nc.gpsimd.tensor_mul(t2[:], rotc[:], sin_sb[:, tp:tp + TC1])
                    nc.vector.tensor_add(dst[h][:, ts:ts + TC1], t1[:], t2[:])
            # v projection: out [t, d] per 128-row t-tile
            for tt in range(TC1 // 128):
                jt = ts // 128 + tt
                ps = pp.tile([128, TC2], F32, name="vps", tag="vr", bufs=2)
                for cc in range(KT):
                    nc.tensor.matmul(
                        ps[:, 0:HPC * D],
                        xt[:, cc * TC1 + tt * 128:cc * TC1 + (tt + 1) * 128],
                        wv_sb[:, cc * HPC * D:(cc + 1) * HPC * D],
                        start=(cc == 0), stop=(cc == KT - 1))
                nc.scalar.copy(vt[jt][:], ps[:, 0:HPC * D])
            # head-0 attention for the chunk this section just completed
            emit_chunk(0, tcn // CI, tcn % CI)
        nc.gpsimd.collective_compute(
            "AllToAll", mybir.AluOpType.bypass,
            replica_groups=[list(range(N_CORES))],
            ins=[a2a_in[0].opt()], outs=[a2a_out[0].opt()])
        # head-1 attention runs after the projections, overlapping the
        # first all-to-all's latency. Largest chunks first: the LAST
        # chunk's exp/normalize/store tail gates the second all-to-all
        # trigger, and the ci=0 chunks have the shortest tails
        for ci in range(CI - 1, -1, -1):
            for b in range(B):
                emit_chunk(1, b, ci)
        nc.gpsimd.collective_compute(
            "AllToAll", mybir.AluOpType.bypass,
            replica_groups=[list(range(N_CORES))],
            ins=[a2a_in[1].opt()], outs=[a2a_out[1].opt()])


def _phase3(nc, tc, wop, wot, a2a_out, y, wha):
    # contraction block g = head s*HPC+k lives in a2a_out[k][s]; group by
    # parity k so the k=1 group alone waits on the last AllToAll
    with tc.tile_pool(name="ao", bufs=1) as aop, \
         tc.tile_pool(name="ps3", bufs=1, space="PSUM") as pp:
        ao = {}
        # trigger issue costs ~600ns each; spread across sync+gpsimd (NOT
        # scalar: the tile scheduler may hoist a trigger between head-1
        # exps, and its collective-wait then blocks the ACT queue --
        # measured 7.6us PE stall via the exp->score pipeline)
        eng_k0 = [nc.sync, nc.gpsimd]
        eng_k1 = [nc.gpsimd, nc.sync]
        for k in range(HPC):
            engs = eng_k0 if k == 0 else eng_k1
            ni = 0
            for s in range(N_CORES):
                g = s * HPC + k
                t_ = aop.tile([128, TC2], F16, name=f"ao{g}")
                for p_ in range(2):
                    engs[ni % len(engs)].dma_start(
                        t_[64 * p_:64 * (p_ + 1), :],
                        a2a_out[k][s, 64 * p_:64 * (p_ + 1), :])
                    ni += 1
                ao[g] = t_
        g0 = [g for g in range(KT) if g % HPC == 0]
        g1 = [g for g in range(KT) if g % HPC != 0]

        def stage_tiles(groups, psname, emit_tail):
            # first column block runs g-outer (arrival order) so the PE
            # starts on the first gathered tile that lands instead of
            # stalling until all 8 are in SBUF; later blocks run tile-major
            for cj in range(C // TC2):
                if cj == 0:
                    yps = [pp.tile([128, TC2], F32, name=psname,
                                   tag=f"{psname}{tt}", bufs=1)
                           for tt in range(TSL // 128)]
                    for n_, g in enumerate(groups):
                        for tt in range(TSL // 128):
                            nc.tensor.matmul(
                                yps[tt][:], ao[g][:, tt * 128:(tt + 1) * 128],
                                wha[cj][:, g * TC2:(g + 1) * TC2],
                                start=(n_ == 0),
                                stop=(n_ == len(groups) - 1))
                    for tt in range(TSL // 128):
                        emit_tail(cj, tt, yps[tt])
                else:
                    for tt in range(TSL // 128):
                        yp = pp.tile([128, TC2], F32, name=psname,
                                     tag=f"{psname}{tt}", bufs=1)
                        for n_, g in enumerate(groups):
                            nc.tensor.matmul(
                                yp[:], ao[g][:, tt * 128:(tt + 1) * 128],
                                wha[cj][:, g * TC2:(g + 1) * TC2],
                                start=(n_ == 0),
                                stop=(n_ == len(groups) - 1))
                        emit_tail(cj, tt, yp)

        # stage A: y_g0 for ALL output tiles, gated only on the first
        # all-to-all -- fills the entire second-collective window with PE work
        ysb0 = {}

        def tail_a(cj, tt, ypa):
            ysb0[cj, tt] = wop.tile([128, TC2], F32, name=f"ys0_{cj}_{tt}")
            if tt % 2 == 0:
                nc.vector.tensor_copy(ysb0[cj, tt][:], ypa[:])
            else:
                nc.scalar.copy(ysb0[cj, tt][:], ypa[:])

        stage_tiles(g0, "ypa", tail_a)

        # stage B: y_g1 (needs the second all-to-all) + add + store.
        # y stores are fp16 (host upcasts; ~5e-4 rel rounding vs a 2e-2
        # gate) -- halves the store bytes so the drain keeps up with the
        # ~2.1us/tile production rate instead of trailing the last matmul
        # by 15us. 4-way partition splits; scalar shares trigger duty (it
        # has no later work in stage B, so a hoisted trigger cannot block).
        yeng = [nc.sync, nc.scalar]
        yi = [0]

        def tail_b(cj, tt, ypb):
            ysb = wop.tile([128, TC2], F16, tag="ysb", bufs=4)
            nc.vector.tensor_add(ysb[:], ypb[:], ysb0[cj, tt][:])
            nsp = 4
            stp = 128 // nsp
            for p_ in range(nsp):
                yeng[yi[0] % len(yeng)].dma_start(
                    y[tt * 128 + p_ * stp:tt * 128 + (p_ + 1) * stp,
                      cj * TC2:(cj + 1) * TC2],
                    ysb[p_ * stp:(p_ + 1) * stp, :])
                yi[0] += 1

        stage_tiles(g1, "ypb", tail_b)


def _build(plan, n_bias):
    nc = bacc.Bacc("TRN2", num_devices=N_CORES)

    # host-pre-tiled inputs: contiguous per-partition runs for fat DMA lines
    xTt = nc.dram_tensor("xTt", [NTC1, 128, KT * TC1], F16,
                         kind="ExternalInput")
    wqt = nc.dram_tensor("wqt", [HPC, 128, KT * D], F16, kind="ExternalInput")
    wkt = nc.dram_tensor("wkt", [HPC, 128, KT * D], F16, kind="ExternalInput")
    wvt = nc.dram_tensor("wvt", [128, KT * HPC * D], F16,
                         kind="ExternalInput")
    wot = nc.dram_tensor("wot", [C // TC2, 128, KT * TC2], F16,
                         kind="ExternalInput")
    cos_d = nc.dram_tensor("cos", [D, T], F16, kind="ExternalInput")
    sin_d = nc.dram_tensor("sin", [D, T], F16, kind="ExternalInput")
    ones_d = nc.dram_tensor("ones", [128, 128], F16, kind="ExternalInput")
    bias_d = nc.dram_tensor("bias", [n_bias, 128, TC2], F16, kind="ExternalInput")
    y = nc.dram_tensor("y", [TSL, C], F16, kind="ExternalOutput")

    with tile.TileContext(nc) as tc:
        with tc.tile_pool(name="const", bufs=1) as cpool, \
             tc.tile_pool(name="dram", bufs=1, space="DRAM") as dram:

            a2a_in = [dram.tile([N_CORES, D, TC2], F16, name=f"a2ai{h}")
                      for h in range(HPC)]
            a2a_out = [dram.tile([N_CORES, D, TC2], F16, name=f"a2ao{h}")
                       for h in range(HPC)]
            warm_in = dram.tile([N_CORES, 1, 2], F16, name="wci")
            warm_out = dram.tile([N_CORES, 1, 2], F16, name="wco")

            with tc.tile_pool(name="qkv", bufs=1) as qkv:
                qT = [qkv.tile([D, BT], F16, name=f"qT{h}") for h in range(HPC)]
                kT = [qkv.tile([D, BT], F16, name=f"kT{h}") for h in range(HPC)]
                vt = [qkv.tile([128, HPC * D], F16, name=f"v{j}")
                      for j in range(BT // 128)]
                wha = {}

                with tc.tile_pool(name="wp", bufs=1) as wp:
                    # only the q-projection weights go ahead of the first x
                    # tile; everything else is issued via late_loads below
                    # split the q weights so the first projection chain can
                    # start consuming chunk 0 after ~one DMA line instead of
                    # waiting for a single 512KB transfer on one queue
                    wq_h = []
                    weng = [nc.sync, nc.scalar, nc.gpsimd]
                    wi = 0
                    for h in range(HPC):
                        w_ = wp.tile([128, KT * D], F16, name=f"wqh{h}")
                        nsp = 4 if h == 0 else 2
                        stp = (KT // nsp) * D
                        for q_ in range(nsp):
                            for p_ in range(2):
                                weng[wi % len(weng)].dma_start(
                                    w_[64 * p_:64 * (p_ + 1),
                                       q_ * stp:(q_ + 1) * stp],
                                    wqt[h, 64 * p_:64 * (p_ + 1),
                                        q_ * stp:(q_ + 1) * stp])
                                wi += 1
                        wq_h.append(w_)
                    wk_h = []
                    cos_sb = wp.tile([D, T], F16)
                    sin_sb = wp.tile([D, T], F16)
                    ones_sb = cpool.tile([128, 128], F16)
                    wv_sb = wp.tile([128, KT * HPC * D], F16)

                    def late_loads():
                        nc.sync.dma_start(cos_sb[:], cos_d[:])
                        nc.scalar.dma_start(sin_sb[:], sin_d[:])
                        HKD = (KT // 2) * D
                        for h in range(HPC):
                            w_ = wp.tile([128, KT * D], F16, name=f"wkh{h}")
                            nc.scalar.dma_start(w_[:, 0:HKD],
                                                wkt[h, :, 0:HKD])
                            nc.gpsimd.dma_start(w_[:, HKD:2 * HKD],
                                                wkt[h, :, HKD:2 * HKD])
                            wk_h.append(w_)
                        nc.sync.dma_start(ones_sb[:], ones_d[:])
                        HVD = (KT // 2) * HPC * D
                        nc.sync.dma_start(wv_sb[:, 0:HVD], wvt[:, 0:HVD])
                        nc.gpsimd.dma_start(wv_sb[:, HVD:2 * HVD],
                                            wvt[:, HVD:2 * HVD])
                        # preload the ACT Exp table so phase 2's first exp
                        # doesn't pay the table switch
                        warm = cpool.tile([128, 1], F32)
                        nc.scalar.activation(warm[:], ones_sb[:, 0:1],
                                             mybir.ActivationFunctionType.Exp,
                                             bias=0.0, scale=1.0)
                        # dummy tiny all-to-all during phase 1 (CC engine is
                        # idle) to absorb the first collective's one-time ring
                        # setup: with it, both real collectives run at a
                        # deterministic ~27us and the head-1 tail always
                        # covers the first one (measured 40-75us without)
                        wct = cpool.tile([8, 2], F16)
                        nc.vector.memset(wct[:], 0.0)
                        nc.sync.dma_start(warm_in[:, 0, :], wct[:])
                        nc.gpsimd.collective_compute(
                            "AllToAll", mybir.AluOpType.bypass,
                            replica_groups=[list(range(N_CORES))],
                            ins=[warm_in.opt()], outs=[warm_out.opt()])

                    _phase12(nc, tc, (qT, kT, vt, wq_h, wk_h, wv_sb),
                             xTt, cos_sb, sin_sb, late_loads,
                             plan, n_bias, bias_d, ones_sb, a2a_in, a2a_out,
                             qkv, wot, wha)

                # wo pool opens as soon as the phase-1 weights are freed so
                # the remaining wo loads stream under the head-1 attention
                with tc.tile_pool(name="wo", bufs=1) as wop:
                    _phase3(nc, tc, wop, wot, a2a_out, y, wha)

    nc.finalize()
    return nc


_cache = {}


def _get_kernel(mask2d):
    key = mask2d.tobytes()
    if key not in _cache:
        plan, bias_tiles = _mask_plan(mask2d)
        nc = _build(plan, bias_tiles.shape[0])
        _cache[key] = (nc, bias_tiles)
    return _cache[key]


def kernel(x, mask, wq, wk, wv, wo, _trace=False):
    x = np.asarray(x)
    mask2d = np.asarray(mask).reshape(T, T).astype(bool)
    nc, bias_tiles = _get_kernel(mask2d)

    cosI, sinI = _rope_tables()
    # [C, BT] -> [NTC1, 128, KT, TC1]: partition p, chunk n <-> row n*128+p
    xT = np.ascontiguousarray(x.reshape(BT, C).T.astype(np.float16))
    xTt = np.ascontiguousarray(
        xT.reshape(KT, 128, NTC1, TC1).transpose(2, 1, 0, 3)).reshape(
            NTC1, 128, KT * TC1)

    def pack_w(w):  # [C, HPC*D] -> [HPC, 128, KT*D]
        w16 = np.asarray(w).astype(np.float16)
        return np.ascontiguousarray(
            w16.reshape(KT, 128, HPC, D).transpose(2, 1, 0, 3)).reshape(
                HPC, 128, KT * D)

    wo16 = np.asarray(wo).astype(np.float16)   # [N_HEADS*D, C]
    wot = np.ascontiguousarray(
        wo16.reshape(KT, 128, C // TC2, TC2).transpose(2, 1, 0, 3)).reshape(
            C // TC2, 128, KT * TC2)

    common = {
        "cos": cosI, "sin": sinI,
        "ones": np.ones((128, 128), np.float16),
        "bias": bias_tiles, "wot": wot, "xTt": xTt,
    }
    in_maps = []
    for c in range(N_CORES):
        sl = slice(c * HPC * D, (c + 1) * HPC * D)
        wv16 = np.asarray(wv)[:, sl].astype(np.float16)
        in_maps.append({
            "wqt": pack_w(np.asarray(wq)[:, sl]),
            "wkt": pack_w(np.asarray(wk)[:, sl]),
            "wvt": np.ascontiguousarray(
                wv16.reshape(KT, 128, HPC * D).transpose(1, 0, 2)).reshape(
                    128, KT * HPC * D),
            **common,
        })

    r = run_bass_kernel_spmd(nc, in_maps, core_ids=list(range(N_CORES)),
                             trace=_trace)
    out = np.empty((BT, C), np.float32)
    for c in range(N_CORES):
        out[c * TSL:(c + 1) * TSL, :] = r.results[c]["y"]
    if _trace:
        kernel.last_results = r
    return out.reshape(B, T, C)



# revision 53
# speedup vs baseline: 1.0179x; 1.0179x over previous
"""Multi-head causal attention (RoPE) forward on 8 Trainium2 NeuronCores.

Sharding: tensor-parallel over heads -- 8 cores x 2 heads, each core handling
both batch elements (the flattened (B*T) = 4096 "time" axis). All matmul
operands are fp16 (PSUM accumulation stays fp32; measured end-to-end rel err
~4e-4 against a 2e-2 gate; fp8 was measured to fail it). Inputs are
host-retiled so every DMA moves contiguous per-partition runs.

Per core:
  merged phase: q/k/v projections in eight 512-column t-sections. RoPE is
      praw*cos + shuffle(praw)*sin with the pair-swap done by a DVE
      stream_shuffle and the signs folded into the host sin table (no PE
      work). Head-0's attention chunk (b, ci) is interleaved right after
      the section that completes its k/q/v, sharing PSUM by tag
      (proj/scores 5 bufs, out-accum 1, v/denominator 2 = 8 banks), so the
      first AllToAll fires at projection end.
  attention (per chunk): scores^T [j, i] = kT^T @ qT with causal
      column-support pruning (diagonal tiles compute only their valid
      column suffix); exp on ScalarE (no max pass -- scores are bounded),
      partial tiles masked by a 0/1 multiply after exp; out^T accumulates
      on PE. The softmax denominator differs by head: head-0 chunks (in
      phase 1, where DVE has slack) accumulate probability tiles on DVE
      and reduce with ONE all-ones ones-matmul per chunk (512 PE rows vs
      ~4.4k); head-1 chunks (under the first AllToAll, where DVE is the
      scarce engine) pair up full-width tiles with one DVE add each so
      the ones-matmul streams once per pair (diagonal tiles stay
      per-tile) -- full accumulation there saturated the DVE and delayed
      the second AllToAll, pairing fits with ~8us DVE slack to spare.
      Normalization uses a fast approximate reciprocal. A
      4-deep score-matmul software pipeline keeps the PE ahead of the exp
      chain. Head-1 runs after the projections largest-chunk-first (the
      last chunk's tail gates the second AllToAll trigger), covering the
      first AllToAll; a tiny warmup collective during the projections
      absorbs the one-time ring setup.
  output projection: y[t-slice, :] = outT_full^T @ wo in two full passes
      split by head parity -- y_g0 (gated only on the first AllToAll)
      computes into SBUF during the second collective's window, then y_g1
      accumulates and adds it back. The first column block of each stage
      runs contraction-group-outer so the PE starts on the first gathered
      tile that lands. y is stored fp16 (host upcasts; ~5e-4 rel rounding
      vs the 2e-2 gate) so the final store drain is ~3us, not ~15us.
Host assembles the 8 t-slices into the full (B, T, C) output.

Hard-won scheduling rules (measured on hardware):
- DMA trigger issue costs ~600ns on an engine queue; only sync/scalar/
  gpsimd may issue them. A trigger whose semaphore waits on a collective
  must NOT sit on a queue with earlier-needed compute: the tile scheduler
  can hoist it and the wait then stalls that engine (7.6us PE stall via a
  blocked ACT exp queue when ao-loads used nc.scalar).
- All DMAs use flattened 2D access patterns (free axis pre-flattened) so
  descriptors carry 2KB+ per-partition runs; 1KB descriptors cap each
  queue at ~10GB/s.
- Do NOT offload bulk tensor ops to gpsimd or oversubscribe DVE: with
  PE+ACT+DVE+gpsimd all hot, every engine slowed ~20% chip-wide
  (power/throttle) and the kernel regressed 25us.
- The dma_start count/order re-rolls the global queue assignment; timings
  move +-5us run to run from this lottery alone.
"""

import os
import sys

import numpy as np

for _p in ("/opt/trn_rl_repo", "/root/.axon_site/_ro/trn_rl_repo"):
    if os.path.isdir(_p) and _p not in sys.path:
        sys.path.append(_p)

import concourse.bacc as bacc
import concourse.tile as tile
from concourse import mybir
from concourse.bass_utils import run_bass_kernel_spmd

B, T, C = 2, 2048, 2048
N_HEADS, D = 16, 128
THETA = 10000.0
N_CORES = 8
HPC = N_HEADS // N_CORES     # heads per core
BT = B * T                   # flattened time axis
TSL = BT // N_CORES          # per-core output slice after the all-to-all
KT = C // 128                # contraction chunks
TC1 = 512                    # phase-1 t-chunk (moving free dim)
NTC1 = BT // TC1
TC2 = 512                    # phase-2/3 chunk
CI = T // TC2                # i-chunks per (head, batch)
JT = T // 128                # j-tiles per (head, batch)
SCALE = 1.0 / np.sqrt(D)
MASKED_BIAS = -1.0e6         # pre-scale units; exp(SCALE*(s+bias)) == 0

F16 = mybir.dt.float16
F32 = mybir.dt.float32
# adjacent-pair swap within each 32-partition quadrant (RoPE rotate-half)
SWAP_MASK = [i + 1 if i % 2 == 0 else i - 1 for i in range(32)]


def _mask_plan(mask2d):
    """Per (ci, jt): None=skip (all masked), (-1, 0)=free (none masked),
    (idx, lo) = partial tile whose columns [lo, TC2) have at least one valid
    row; idx is a 0/1 multiplier tile applied to exp(scores) on that column
    range. scoresT tile (jt, ci) holds mask2d[i, j] transposed:
    m01[j_loc, i_loc] <- mask2d[TC2*ci+i, 128*jt+j].
    """
    uniq = {}
    tiles = []
    plan = []
    for ci in range(CI):
        row = []
        for jt in range(JT):
            blk = mask2d[TC2 * ci:TC2 * (ci + 1), 128 * jt:128 * (jt + 1)]
            if blk.all():
                row.append((-1, 0))
            elif not blk.any():
                row.append(None)
            else:
                m01 = blk.T.astype(np.float16)   # [128 j, TC2 i]
                lo = int(np.argmax(m01.any(axis=0)))
                key = m01.tobytes()
                if key not in uniq:
                    uniq[key] = len(tiles)
                    tiles.append(m01)
                row.append((uniq[key], lo))
        plan.append(row)
    if not tiles:  # keep the DRAM tensor non-empty
        tiles.append(np.zeros((128, TC2), np.float16))
    return plan, np.stack(tiles)


def _rope_tables():
    inv_freq = 1.0 / (THETA ** (np.arange(0, D, 2, dtype=np.float64) / D))
    freqs = np.outer(inv_freq, np.arange(T, dtype=np.float64))  # [64, T]
    cosI = np.repeat(np.cos(freqs), 2, axis=0).astype(np.float16)  # [128, T]
    # the pair-swap runs as a signless DVE stream_shuffle, so the RoPE signs
    # ride in the sin table: row 2i gets -sin (multiplies x[2i+1]), row
    # 2i+1 gets +sin (multiplies x[2i])
    sinI = np.repeat(np.sin(freqs), 2, axis=0).astype(np.float16)
    sinI[0::2, :] *= np.float16(-1.0)
    return cosI, sinI


def _phase12(nc, tc, qkv_tensors, xTt, cos_sb, sin_sb, late_loads,
             plan, n_bias, bias_d, ones_sb, a2a_in, a2a_out, qkv, wot, wha):
    """Projections with head-0 attention chunks interleaved: chunk
    (h=0, b, ci) only needs k/q/v through t-chunk 4*b+ci, so it runs right
    after that section and the first all-to-all fires at projection end
    instead of mid-attention. PSUM is shared by tag: proj/sc (4 bufs),
    rot/outT (2), vps/r (2) = exactly 8 banks."""
    qT, kT, vt, wq_h, wk_h, wv_sb = qkv_tensors
    with tc.tile_pool(name="xt", bufs=2) as xp, \
         tc.tile_pool(name="p1t", bufs=1) as p1, \
         tc.tile_pool(name="p2t", bufs=1) as p2, \
         tc.tile_pool(name="ps12", bufs=1, space="PSUM") as pp:
        mask_sb = p2.tile([128, n_bias, TC2], F16)
        LA = 4   # sc-matmul lookahead so the PE never waits on ACT's exp

        def emit_chunk(h, b, ci, last=False):
            gci = b * CI + ci      # global chunk == dest rank
            live = [(jt, plan[ci][jt]) for jt in range(JT)
                    if plan[ci][jt] is not None]
            if not live:
                z = p2.tile([128, TC2], F16, name="z", tag="ot", bufs=3)
                nc.vector.memset(z[:], 0.0)
                nc.sync.dma_start(a2a_in[h][gci, :, :], z[:])
                return
            # column-support pruning relies on the first live tile
            # starting the full accumulation region
            full_width = live[0][1][1] != 0
            outp = pp.tile([D, TC2], F32, name="outp", tag="mid", bufs=1)
            i0 = b * T + ci * TC2
            nlive = len(live)
            pend = [None] * nlive
            # Denominator, two flavors. Head-0 chunks run inside phase 1
            # where the DVE has slack: probability tiles accumulate on DVE
            # (fp16 adds) and ONE ones-matmul per chunk reduces over j
            # (512 PE rows instead of ~4.4k). Head-1 chunks run under the
            # first all-to-all where the DVE is the scarce engine and PE
            # has slack: keep the per-tile PE ones-matmul so the chunk
            # tails (-> ot DMA -> second all-to-all trigger) stay early.
            # (Tried: head-1 on DVE too, normalize on gpsimd -- every
            # engine slowed ~20% chip-wide and a2a#2 slipped 28us.)
            dve_acc = h == 0
            acc = None
            rp = pp.tile([128, TC2], F32, name="rp", tag="vr", bufs=2)
            if dve_acc:
                acc = p2.tile([128, TC2], F16, name="acc", tag="acc",
                              bufs=3)
            # head-1 middle ground: pair up the full-width tiles on DVE
            # (one add each) so the PE ones-matmul streams once per PAIR;
            # diagonal tiles keep the per-tile matmul. ~6us less PE in the
            # window that gates the second all-to-all, and the extra DVE
            # fits (unlike full accumulation, which saturated it).
            nf = 0
            if not dve_acc and not full_width:
                while (nf < nlive and live[nf][1][0] < 0
                       and live[nf][1][1] == 0):
                    nf += 1
                nf -= nf % 4   # quads only; leftovers take the diag path
            paccs = {}
            for step in range(nlive + LA):
                if step < nlive:
                    jt, (code, lo) = live[step]
                    if full_width:
                        lo = 0
                    sc = pp.tile([128, TC2], F32, name="sc", tag="big",
                                 bufs=5)
                    nc.tensor.matmul(
                        sc[:, lo:],
                        kT[h][:, b * T + jt * 128:b * T + (jt + 1) * 128],
                        qT[h][:, i0 + lo:i0 + TC2], start=True, stop=True)
                    pt = p2.tile([128, TC2], F16, name="pt", tag="pt",
                                 bufs=5)
                    nc.scalar.activation(
                        pt[:, lo:], sc[:, lo:],
                        mybir.ActivationFunctionType.Exp,
                        bias=0.0, scale=float(SCALE))
                    if code >= 0:
                        ptm = p2.tile([128, TC2], F16, name="ptm",
                                      tag="ptm", bufs=4)
                        nc.vector.tensor_mul(ptm[:, lo:], pt[:, lo:],
                                             mask_sb[:, code, lo:])
                        pt = ptm
                    if dve_acc:
                        if step == 0:
                            nc.vector.tensor_copy(acc[:, lo:], pt[:, lo:])
                        else:
                            nc.vector.tensor_add(acc[:, lo:], acc[:, lo:],
                                                 pt[:, lo:])
                    elif step < nf and step % 2 == 1:
                        # quad tree on DVE: two pair adds + one in-place
                        # root add per 4 full tiles; the ring shares the
                        # (phase-disjoint) acc tag
                        pacc = p2.tile([128, TC2], F16, name="pacc",
                                       tag="acc", bufs=3)
                        nc.vector.tensor_add(pacc[:], pend[step - 1][0][:],
                                             pt[:])
                        if step % 4 == 1:
                            paccs[step - 1] = pacc
                        else:
                            root = paccs[step - 3]
                            nc.vector.tensor_add(root[:], root[:], pacc[:])
                    pend[step] = (pt, lo)
                j = step - LA
                if 0 <= j < nlive:
                    pt_j, lo_j = pend[j]
                    pend[j] = None
                    jv = (b * T) // 128 + live[j][0]
                    nc.tensor.matmul(
                        outp[:, lo_j:],
                        vt[jv][:, h * D:(h + 1) * D], pt_j[:, lo_j:],
                        start=(j == 0), stop=(j == nlive - 1))
                    if not dve_acc:
                        # ones_sb is [128, 128] all-ones: every partition
                        # of rp gets the denominator -- broadcast free on PE
                        if j < nf:
                            if j % 4 == 3:
                                nc.tensor.matmul(
                                    rp[:], ones_sb[:], paccs.pop(j - 3)[:],
                                    start=(j == 3), stop=(j == nlive - 1))
                        else:
                            nc.tensor.matmul(
                                rp[:, lo_j:], ones_sb[:], pt_j[:, lo_j:],
                                start=(j == 0 and nf == 0),
                                stop=(j == nlive - 1))
            if dve_acc:
                nc.tensor.matmul(rp[:], ones_sb[:], acc[:],
                                 start=True, stop=True)
            ri = p2.tile([128, TC2], F32, name="ri", tag="ri", bufs=1)
            # ~18 correct bits, 5x faster than reciprocal(); r >= exp(s_ii)
            # here so the undefined edge cases cannot occur
            nc.vector.reciprocal_approx_fast(ri[:], rp[:])
            ot = p2.tile([128, TC2], F16, name="ot", tag="ot", bufs=3)
            if last:
                # final chunk before the PSUM pool handoff to phase 3:
                # evacuate outp via the idle ACT so the bank's last reader
                # retires ~0.7us earlier (the pool-open barrier waits on it)
                osb = p2.tile([128, TC2], F16, name="osb", tag="acc",
                              bufs=3)
                nc.scalar.copy(osb[:], outp[:])
                nc.vector.tensor_mul(ot[:], osb[:], ri[:])
            else:
                nc.vector.tensor_mul(ot[:], outp[:], ri[:])
            # 4-way split: the last chunk's ot store gates the collective
            # trigger; 32KB per queue lands ~3us sooner than 64KB
            for p_ in range(4):
                nc.sync.dma_start(a2a_in[h][gci, 32 * p_:32 * (p_ + 1), :],
                                  ot[32 * p_:32 * (p_ + 1), :])

        # startup triggers spread over engines idle at t=0 (trigger issue
        # is ~600ns each; one queue serializes the whole first-tile load)
        eng0 = [nc.sync, nc.scalar, nc.gpsimd]
        for tcn in range(NTC1):
            ts = tcn * TC1           # position in flattened BT
            tp = ts % T              # rope position (restarts per batch)
            # flattened free axis: per-partition contiguous 2KB+ runs give
            # the DMA engines full-size descriptors instead of 1KB rows
            xt = xp.tile([128, KT * TC1], F16, tag="xt")
            nparts = 16 if tcn == 0 else 2
            step = (KT // nparts) * TC1
            for q_ in range(nparts):
                if tcn == 0:
                    # first slice 4-way: the very first matmul waits on it
                    psplit = 4 if q_ == 0 else 2
                    pstep = 128 // psplit
                    for p_ in range(psplit):
                        eng0[(2 * q_ + p_) % len(eng0)].dma_start(
                            xt[pstep * p_:pstep * (p_ + 1),
                               q_ * step:(q_ + 1) * step],
                            xTt[tcn, pstep * p_:pstep * (p_ + 1),
                                q_ * step:(q_ + 1) * step])
                else:
                    nc.sync.dma_start(xt[:, q_ * step:(q_ + 1) * step],
                                      xTt[tcn, :, q_ * step:(q_ + 1) * step])
            if tcn == 0:
                late_loads()     # weight DMAs behind the critical first loads
                nc.sync.dma_start(mask_sb[:],
                                  bias_d.rearrange("u p m -> p u m"))
            if tcn in (NTC1 // 2, NTC1 // 2 + 1):
                # prefetch the output-projection weights in the DMA-idle
                # mid-projection window, so no wo traffic contends with the
                # first all-to-all's wire phase
                for cj in (0, 1) if tcn == NTC1 // 2 else (2, 3):
                    wha[cj] = qkv.tile([128, KT * TC2], F16, name=f"wo{cj}")
                    stp = 4 * TC2
                    for q_ in range(4):
                        nc.sync.dma_start(
                            wha[cj][:, q_ * stp:(q_ + 1) * stp],
                            wot[cj, :, q_ * stp:(q_ + 1) * stp])
            for dst, w_h in ((qT, wq_h), (kT, wk_h)):
                for h in range(HPC):
                    ps = pp.tile([D, TC1], F32, name="ps", tag="big", bufs=5)
                    for cc in range(KT):
                        nc.tensor.matmul(
                            ps[:], w_h[h][:, cc * D:(cc + 1) * D],
                            xt[:, cc * TC1:(cc + 1) * TC1],
                            start=(cc == 0), stop=(cc == KT - 1))
                    praw = p1.tile([D, TC1], F16, tag="praw", bufs=3)
                    nc.scalar.copy(praw[:], ps[:])
                    rotc = p1.tile([D, TC1], F16, tag="rotc", bufs=2)
                    nc.vector.stream_shuffle(rotc[:], praw[:], SWAP_MASK)
                    t1 = p1.tile([D, TC1], F16, tag="t1", bufs=2)
                    nc.vector.tensor_mul(t1[:], praw[:], cos_sb[:, tp:tp + TC1])
                    t2 = p1.tile([D, TC1], F16, tag="t2", bufs=2)
                    nc.gpsimd.tensor_mul(t2[:], rotc[:], sin_sb[:, tp:tp + TC1])
                    nc.vector.tensor_add(dst[h][:, ts:ts + TC1], t1[:], t2[:])
            # v projection: out [t, d] per 128-row t-tile
            for tt in range(TC1 // 128):
                jt = ts // 128 + tt
                ps = pp.tile([128, TC2], F32, name="vps", tag="vr", bufs=2)
                for cc in range(KT):
                    nc.tensor.matmul(
                        ps[:, 0:HPC * D],
                        xt[:, cc * TC1 + tt * 128:cc * TC1 + (tt + 1) * 128],
                        wv_sb[:, cc * HPC * D:(cc + 1) * HPC * D],
                        start=(cc == 0), stop=(cc == KT - 1))
                nc.scalar.copy(vt[jt][:], ps[:, 0:HPC * D])
            # head-0 attention for the chunk this section just completed
            emit_chunk(0, tcn // CI, tcn % CI)
        nc.gpsimd.collective_compute(
            "AllToAll", mybir.AluOpType.bypass,
            replica_groups=[list(range(N_CORES))],
            ins=[a2a_in[0].opt()], outs=[a2a_out[0].opt()])
        # head-1 attention runs after the projections, overlapping the
        # first all-to-all's latency. Largest chunks first: the LAST
        # chunk's exp/normalize/store tail gates the second all-to-all
        # trigger, and the ci=0 chunks have the shortest tails
        for ci in range(CI - 1, -1, -1):
            for b in range(B):
                emit_chunk(1, b, ci, last=(ci == 0 and b == B - 1))
        nc.gpsimd.collective_compute(
            "AllToAll", mybir.AluOpType.bypass,
            replica_groups=[list(range(N_CORES))],
            ins=[a2a_in[1].opt()], outs=[a2a_out[1].opt()])


def _phase3(nc, tc, wop, wot, a2a_out, y, wha):
    # contraction block g = head s*HPC+k lives in a2a_out[k][s]; group by
    # parity k so the k=1 group alone waits on the last AllToAll
    with tc.tile_pool(name="ao", bufs=1) as aop, \
         tc.tile_pool(name="ps3", bufs=1, space="PSUM") as pp:
        ao = {}
        # trigger issue costs ~600ns each; spread across sync+gpsimd (NOT
        # scalar: the tile scheduler may hoist a trigger between head-1
        # exps, and its collective-wait then blocks the ACT queue --
        # measured 7.6us PE stall via the exp->score pipeline)
        eng_k0 = [nc.sync, nc.gpsimd]
        eng_k1 = [nc.gpsimd, nc.sync]
        for k in range(HPC):
            engs = eng_k0 if k == 0 else eng_k1
            ni = 0
            for s in range(N_CORES):
                g = s * HPC + k
                t_ = aop.tile([128, TC2], F16, name=f"ao{g}")
                for p_ in range(2):
                    engs[ni % len(engs)].dma_start(
                        t_[64 * p_:64 * (p_ + 1), :],
                        a2a_out[k][s, 64 * p_:64 * (p_ + 1), :])
                    ni += 1
                ao[g] = t_
        g0 = [g for g in range(KT) if g % HPC == 0]
        g1 = [g for g in range(KT) if g % HPC != 0]

        def stage_tiles(groups, psname, emit_tail):
            # first column block runs g-outer (arrival order) so the PE
            # starts on the first gathered tile that lands instead of
            # stalling until all 8 are in SBUF; later blocks run tile-major
            for cj in range(C // TC2):
                if cj == 0:
                    yps = [pp.tile([128, TC2], F32, name=psname,
                                   tag=f"{psname}{tt}", bufs=1)
                           for tt in range(TSL // 128)]
                    for n_, g in enumerate(groups):
                        for tt in range(TSL // 128):
                            nc.tensor.matmul(
                                yps[tt][:], ao[g][:, tt * 128:(tt + 1) * 128],
                                wha[cj][:, g * TC2:(g + 1) * TC2],
                                start=(n_ == 0),
                                stop=(n_ == len(groups) - 1))
                    for tt in range(TSL // 128):
                        emit_tail(cj, tt, yps[tt])
                else:
                    for tt in range(TSL // 128):
                        yp = pp.tile([128, TC2], F32, name=psname,
                                     tag=f"{psname}{tt}", bufs=1)
                        for n_, g in enumerate(groups):
                            nc.tensor.matmul(
                                yp[:], ao[g][:, tt * 128:(tt + 1) * 128],
                                wha[cj][:, g * TC2:(g + 1) * TC2],
                                start=(n_ == 0),
                                stop=(n_ == len(groups) - 1))
                        emit_tail(cj, tt, yp)

        # stage A: y_g0 for ALL output tiles, gated only on the first
        # all-to-all -- fills the entire second-collective window with PE work
        ysb0 = {}

        def tail_a(cj, tt, ypa):
            ysb0[cj, tt] = wop.tile([128, TC2], F32, name=f"ys0_{cj}_{tt}")
            if tt % 2 == 0:
                nc.vector.tensor_copy(ysb0[cj, tt][:], ypa[:])
            else:
                nc.scalar.copy(ysb0[cj, tt][:], ypa[:])

        stage_tiles(g0, "ypa", tail_a)

        # stage B: y_g1 (needs the second all-to-all) + add + store.
        # y stores are fp16 (host upcasts; ~5e-4 rel rounding vs a 2e-2
        # gate) -- halves the store bytes so the drain keeps up with the
        # ~2.1us/tile production rate instead of trailing the last matmul
        # by 15us. 4-way partition splits; scalar shares trigger duty (it
        # has no later work in stage B, so a hoisted trigger cannot block).
        yeng = [nc.sync, nc.scalar]
        yi = [0]

        def tail_b(cj, tt, ypb):
            ysb = wop.tile([128, TC2], F16, tag="ysb", bufs=4)
            nc.vector.tensor_add(ysb[:], ypb[:], ysb0[cj, tt][:])
            nsp = 4
            stp = 128 // nsp
            for p_ in range(nsp):
                yeng[yi[0] % len(yeng)].dma_start(
                    y[tt * 128 + p_ * stp:tt * 128 + (p_ + 1) * stp,
                      cj * TC2:(cj + 1) * TC2],
                    ysb[p_ * stp:(p_ + 1) * stp, :])
                yi[0] += 1

        stage_tiles(g1, "ypb", tail_b)


def _build(plan, n_bias):
    nc = bacc.Bacc("TRN2", num_devices=N_CORES)

    # host-pre-tiled inputs: contiguous per-partition runs for fat DMA lines
    xTt = nc.dram_tensor("xTt", [NTC1, 128, KT * TC1], F16,
                         kind="ExternalInput")
    wqt = nc.dram_tensor("wqt", [HPC, 128, KT * D], F16, kind="ExternalInput")
    wkt = nc.dram_tensor("wkt", [HPC, 128, KT * D], F16, kind="ExternalInput")
    wvt = nc.dram_tensor("wvt", [128, KT * HPC * D], F16,
                         kind="ExternalInput")
    wot = nc.dram_tensor("wot", [C // TC2, 128, KT * TC2], F16,
                         kind="ExternalInput")
    cos_d = nc.dram_tensor("cos", [D, T], F16, kind="ExternalInput")
    sin_d = nc.dram_tensor("sin", [D, T], F16, kind="ExternalInput")
    ones_d = nc.dram_tensor("ones", [128, 128], F16, kind="ExternalInput")
    bias_d = nc.dram_tensor("bias", [n_bias, 128, TC2], F16, kind="ExternalInput")
    y = nc.dram_tensor("y", [TSL, C], F16, kind="ExternalOutput")

    with tile.TileContext(nc) as tc:
        with tc.tile_pool(name="const", bufs=1) as cpool, \
             tc.tile_pool(name="dram", bufs=1, space="DRAM") as dram:

            a2a_in = [dram.tile([N_CORES, D, TC2], F16, name=f"a2ai{h}")
                      for h in range(HPC)]
            a2a_out = [dram.tile([N_CORES, D, TC2], F16, name=f"a2ao{h}")
                       for h in range(HPC)]
            warm_in = dram.tile([N_CORES, 1, 2], F16, name="wci")
            warm_out = dram.tile([N_CORES, 1, 2], F16, name="wco")

            with tc.tile_pool(name="qkv", bufs=1) as qkv:
                qT = [qkv.tile([D, BT], F16, name=f"qT{h}") for h in range(HPC)]
                kT = [qkv.tile([D, BT], F16, name=f"kT{h}") for h in range(HPC)]
                vt = [qkv.tile([128, HPC * D], F16, name=f"v{j}")
                      for j in range(BT // 128)]
                wha = {}

                with tc.tile_pool(name="wp", bufs=1) as wp:
                    # only the q-projection weights go ahead of the first x
                    # tile; everything else is issued via late_loads below
                    # split the q weights so the first projection chain can
                    # start consuming chunk 0 after ~one DMA line instead of
                    # waiting for a single 512KB transfer on one queue
                    wq_h = []
                    weng = [nc.sync, nc.scalar, nc.gpsimd]
                    wi = 0
                    for h in range(HPC):
                        w_ = wp.tile([128, KT * D], F16, name=f"wqh{h}")
                        nsp = 4 if h == 0 else 2
                        stp = (KT // nsp) * D
                        for q_ in range(nsp):
                            for p_ in range(2):
                                weng[wi % len(weng)].dma_start(
                                    w_[64 * p_:64 * (p_ + 1),
                                       q_ * stp:(q_ + 1) * stp],
                                    wqt[h, 64 * p_:64 * (p_ + 1),
                                        q_ * stp:(q_ + 1) * stp])
                                wi += 1
                        wq_h.append(w_)
                    wk_h = []
                    cos_sb = wp.tile([D, T], F16)
                    sin_sb = wp.tile([D, T], F16)
                    ones_sb = cpool.tile([128, 128], F16)
                    wv_sb = wp.tile([128, KT * HPC * D], F16)

                    def late_loads():
                        nc.sync.dma_start(cos_sb[:], cos_d[:])
                        nc.scalar.dma_start(sin_sb[:], sin_d[:])
                        HKD = (KT // 2) * D
                        for h in range(HPC):
                            w_ = wp.tile([128, KT * D], F16, name=f"wkh{h}")
                            nc.scalar.dma_start(w_[:, 0:HKD],
                                                wkt[h, :, 0:HKD])
                            nc.gpsimd.dma_start(w_[:, HKD:2 * HKD],
                                                wkt[h, :, HKD:2 * HKD])
                            wk_h.append(w_)
                        nc.sync.dma_start(ones_sb[:], ones_d[:])
                        HVD = (KT // 2) * HPC * D
                        nc.sync.dma_start(wv_sb[:, 0:HVD], wvt[:, 0:HVD])
                        nc.gpsimd.dma_start(wv_sb[:, HVD:2 * HVD],
                                            wvt[:, HVD:2 * HVD])
                        # preload the ACT Exp table so phase 2's first exp
                        # doesn't pay the table switch
                        warm = cpool.tile([128, 1], F32)
                        nc.scalar.activation(warm[:], ones_sb[:, 0:1],
                                             mybir.ActivationFunctionType.Exp,
                                             bias=0.0, scale=1.0)
                        # dummy tiny all-to-all during phase 1 (CC engine is
                        # idle) to absorb the first collective's one-time ring
                        # setup: with it, both real collectives run at a
                        # deterministic ~27us and the head-1 tail always
                        # covers the first one (measured 40-75us without)
                        wct = cpool.tile([8, 2], F16)
                        nc.vector.memset(wct[:], 0.0)
                        nc.sync.dma_start(warm_in[:, 0, :], wct[:])
                        nc.gpsimd.collective_compute(
                            "AllToAll", mybir.AluOpType.bypass,
                            replica_groups=[list(range(N_CORES))],
                            ins=[warm_in.opt()], outs=[warm_out.opt()])

                    _phase12(nc, tc, (qT, kT, vt, wq_h, wk_h, wv_sb),
                             xTt, cos_sb, sin_sb, late_loads,
                             plan, n_bias, bias_d, ones_sb, a2a_in, a2a_out,
                             qkv, wot, wha)

                # wo pool opens as soon as the phase-1 weights are freed so
                # the remaining wo loads stream under the head-1 attention
                with tc.tile_pool(name="wo", bufs=1) as wop:
                    _phase3(nc, tc, wop, wot, a2a_out, y, wha)

    nc.finalize()
    return nc


_cache = {}


def _get_kernel(mask2d):
    key = mask2d.tobytes()
    if key not in _cache:
        plan, bias_tiles = _mask_plan(mask2d)
        nc = _build(plan, bias_tiles.shape[0])
        _cache[key] = (nc, bias_tiles)
    return _cache[key]


def kernel(x, mask, wq, wk, wv, wo, _trace=False):
    x = np.asarray(x)
    mask2d = np.asarray(mask).reshape(T, T).astype(bool)
    nc, bias_tiles = _get_kernel(mask2d)

    cosI, sinI = _rope_tables()
    # [C, BT] -> [NTC1, 128, KT, TC1]: partition p, chunk n <-> row n*128+p
    xT = np.ascontiguousarray(x.reshape(BT, C).T.astype(np.float16))
    xTt = np.ascontiguousarray(
        xT.reshape(KT, 128, NTC1, TC1).transpose(2, 1, 0, 3)).reshape(
            NTC1, 128, KT * TC1)

    def pack_w(w):  # [C, HPC*D] -> [HPC, 128, KT*D]
        w16 = np.asarray(w).astype(np.float16)
        return np.ascontiguousarray(
            w16.reshape(KT, 128, HPC, D).transpose(2, 1, 0, 3)).reshape(
                HPC, 128, KT * D)

    wo16 = np.asarray(wo).astype(np.float16)   # [N_HEADS*D, C]
    wot = np.ascontiguousarray(
        wo16.reshape(KT, 128, C // TC2, TC2).transpose(2, 1, 0, 3)).reshape(
            C // TC2, 128, KT * TC2)

    common = {
        "cos": cosI, "sin": sinI,
        "ones": np.ones((128, 128), np.float16),
        "bias": bias_tiles, "wot": wot, "xTt": xTt,
    }
    in_maps = []
    for c in range(N_CORES):
        sl = slice(c * HPC * D, (c + 1) * HPC * D)
        wv16 = np.asarray(wv)[:, sl].astype(np.float16)
        in_maps.append({
            "wqt": pack_w(np.asarray(wq)[:, sl]),
            "wkt": pack_w(np.asarray(wk)[:, sl]),
            "wvt": np.ascontiguousarray(
                wv16.reshape(KT, 128, HPC * D).transpose(1, 0, 2)).reshape(
                    128, KT * HPC * D),
            **common,
        })

    r = run_bass_kernel_spmd(nc, in_maps, core_ids=list(range(N_CORES)),
                             trace=_trace)
    out = np.empty((BT, C), np.float32)
    for c in range(N_CORES):
        out[c * TSL:(c + 1) * TSL, :] = r.results[c]["y"]
    if _trace:
        kernel.last_results = r
    return out.reshape(B, T, C)

